# revision 3
# baseline (speedup 1.0000x reference)
"""Sum-reduced BCE-with-logits loss on 8 Trainium2 NeuronCores.

reference: loss = sum(softplus(x) - x * (labels > 0))  over x[1e6, 23] f32.

Strategy (data-parallel, per sharding hint):
  - Flatten x/target to 23M elements, pad to 8*128*22464, shard rows across
    8 cores; each core sees x_d [128, 22464] f32 and t_d [128, 22464] bf16.
  - Per core, stream 6 tiles of [128, 3744]:
      ACT: softplus(x) with per-partition accumulate (one pass)
      DVE: tensor_tensor_reduce accumulates -x*t  (one pass)
  - Finish: reduce partials to [128,1], cross-partition sum via PE matmul
    with a ones vector -> scalar per core; host adds the 8 scalars.
Device time is DMA-bound: ~17.25 MB/core over ~358 GB/s.
"""

import numpy as np

P = 128          # SBUF partitions
F = 22464        # per-core free dim (8*128*22464 = 23,003,136 >= 23e6)
FT = 3744        # tile free dim  (F = 6 * FT)
NT = 6           # tiles per core
NCORES = 8
TOTAL = 23_000_000
TOTAL_PAD = NCORES * P * F
X_PAD = -30.0    # softplus(-30) ~ 9.4e-14: invisible in the sum

_cache = {}


def _build_nc(softplus_native=True):
    import concourse.bacc as bacc
    import concourse.mybir as mybir
    from concourse import tile

    f32 = mybir.dt.float32
    bf16 = mybir.dt.bfloat16
    AF = mybir.ActivationFunctionType
    ALU = mybir.AluOpType

    nc = bacc.Bacc("TRN2", target_bir_lowering=False, debug=False)
    x_d = nc.dram_tensor("x", [P, F], f32, kind="ExternalInput")
    t_d = nc.dram_tensor("t", [P, F], bf16, kind="ExternalInput")
    o_d = nc.dram_tensor("o", [1, 1], f32, kind="ExternalOutput")

    with tile.TileContext(nc) as tc:
        with (
            tc.tile_pool(name="xin", bufs=3) as xpool,
            tc.tile_pool(name="tin", bufs=3) as tpool,
            tc.tile_pool(name="junk", bufs=2) as jpool,
            tc.tile_pool(name="stats", bufs=1) as spool,
            tc.tile_pool(name="psum", bufs=1, space="PSUM") as ppool,
        ):
            acc_sp = spool.tile([P, NT], f32)   # ACT-only partials
            acc_xt = spool.tile([P, NT], f32)   # DVE-only partials (-x*t)
            for i in range(NT):
                x_t = xpool.tile([P, FT], f32, tag="x")
                t_t = tpool.tile([P, FT], bf16, tag="t")
                nc.sync.dma_start(out=x_t[:], in_=x_d[:, i * FT:(i + 1) * FT])
                nc.sync.dma_start(out=t_t[:], in_=t_d[:, i * FT:(i + 1) * FT])

                sp_junk = jpool.tile([P, FT], f32, tag="spj")
                if softplus_native:
                    nc.scalar.activation(
                        sp_junk[:], x_t[:], AF.Softplus,
                        accum_out=acc_sp[:, i:i + 1],
                    )
                else:
                    # CoreSim doesn't model Softplus; ln(exp(x) + 1) variant
                    # (inputs are N(0,1): exp never overflows).
                    e_junk = jpool.tile([P, FT], f32, tag="ej")
                    nc.scalar.activation(e_junk[:], x_t[:], AF.Exp)
                    nc.scalar.activation(
                        sp_junk[:], e_junk[:], AF.Ln, bias=1.0,
                        accum_out=acc_sp[:, i:i + 1],
                    )

                # InstTensorTensorReduce crashes this runtime (NRT INTERNAL);
                # scalar_tensor_tensor gives the same one-pass mul+sum:
                # out = (x * -1) * t, accum = sum(out) = -sum(x*t).
                tt_junk = jpool.tile([P, FT], f32, tag="ttj")
                nc.vector.scalar_tensor_tensor(
                    out=tt_junk[:], in0=x_t[:], scalar=-1.0, in1=t_t[:],
                    op0=ALU.mult, op1=ALU.mult,
                    accum_out=acc_xt[:, i:i + 1],
                )

            r_sp = spool.tile([P, 1], f32)
            r_xt = spool.tile([P, 1], f32)
            nc.vector.tensor_reduce(
                out=r_sp[:], in_=acc_sp[:], axis=mybir.AxisListType.X, op=ALU.add)
            nc.vector.tensor_reduce(
                out=r_xt[:], in_=acc_xt[:], axis=mybir.AxisListType.X, op=ALU.add)
            total = spool.tile([P, 1], f32)
            nc.vector.tensor_add(total[:], r_sp[:], r_xt[:])

            ones = spool.tile([P, 1], f32)
            nc.vector.memset(ones[:], 1.0)
            ps = ppool.tile([1, 1], f32)
            nc.tensor.matmul(ps[:], total[:], ones[:], start=True, stop=True)
            res = spool.tile([1, 1], f32)
            nc.vector.tensor_copy(res[:], ps[:])
            nc.sync.dma_start(out=o_d[:], in_=res[:])

    nc.compile()
    return nc


def _get_nc(softplus_native=True):
    key = ("nc", softplus_native)
    if key not in _cache:
        _cache[key] = _build_nc(softplus_native)
    return _cache[key]


def _prep(x, labels):
    import ml_dtypes
    bf16 = np.dtype(ml_dtypes.bfloat16)
    x = np.asarray(x, dtype=np.float32).reshape(-1)
    t = (np.asarray(labels).reshape(-1) > 0)

    xf = np.full(TOTAL_PAD, X_PAD, dtype=np.float32)
    xf[:TOTAL] = x
    tf = np.zeros(TOTAL_PAD, dtype=bf16)
    tf[:TOTAL] = t.astype(bf16)
    return xf.reshape(NCORES, P, F), tf.reshape(NCORES, P, F)


def kernel(x, labels, _trace=False, _softplus_native=False):
    # NOTE: native Softplus is not in this toolchain's ACT tables; the
    # exp+ln variant uses the natural_log_exp_and_others set (one load).
    from concourse.bass_utils import run_bass_kernel_spmd

    xs, ts = _prep(x, labels)
    nc = _get_nc(_softplus_native)
    in_maps = [{"x": xs[c], "t": ts[c]} for c in range(NCORES)]
    r = run_bass_kernel_spmd(nc, in_maps, list(range(NCORES)), trace=_trace)
    total = sum(float(r.results[c]["o"][0, 0]) for c in range(NCORES))
    out = np.asarray(total, dtype=np.float32)
    if _trace:
        _cache["last_results"] = r
    return out


# revision 4
# speedup vs baseline: 1.0950x; 1.0950x over previous
"""Sum-reduced BCE-with-logits loss on 8 Trainium2 NeuronCores.

reference: loss = sum(softplus(x) - x * (labels > 0))  over x[1e6, 23] f32.

Strategy (data-parallel, per sharding hint):
  - Flatten x/target to 23M elements, pad to 8*128*22464, shard rows across
    8 cores; core c sees x_d [128, 22464] bf16 and t_d [128, 22464] fp8e4.
    (bf16 x changes the final sum by ~1.5e-8 relative — rounding cancels
    over 23M terms; fp8 {0,1} targets are exact.)
  - Per core, stream 7 chunks (two 1872-wide lead-ins to warm the pipe,
    then 3744-wide):
      ACT: exp(x) then ln(1+u) [softplus; this toolchain has no native
           softplus table, natural_log_exp_and_others has exp+ln] with
           per-partition accumulate on the ln.
      DVE: scalar_tensor_tensor accumulates -(x*t) in one pass.
    x loads ride HWDGE (nc.sync), t loads ride SWDGE (nc.gpsimd) so the
    two streams don't serialize on one FIFO.
  - Finish: reduce partials to [128,1], cross-partition sum via PE matmul
    with a ones vector -> scalar per core; host adds the 8 scalars.
Device time ~= ACT bound: 2 passes over 2.88M elem/core @153.6 G elem/s.
"""

import numpy as np

P = 128          # SBUF partitions
F = 22464        # per-core free dim (8*128*22464 = 23,003,136 >= 23e6)
CHUNKS = [1872, 1872, 3744, 3744, 3744, 3744, 3744]   # sum == F
NCORES = 8
TOTAL = 23_000_000
TOTAL_PAD = NCORES * P * F
X_PAD = -30.0    # exp(-30) ~ 9e-14; ln(1+u) == 0.0 in f32

assert sum(CHUNKS) == F

_cache = {}


def _build_nc():
    import concourse.bacc as bacc
    import concourse.mybir as mybir
    from concourse import tile

    f32 = mybir.dt.float32
    bf16 = mybir.dt.bfloat16
    fp8 = mybir.dt.float8e4
    AF = mybir.ActivationFunctionType
    ALU = mybir.AluOpType

    nc = bacc.Bacc("TRN2", target_bir_lowering=False, debug=False)
    x_d = nc.dram_tensor("x", [P, F], bf16, kind="ExternalInput")
    t_d = nc.dram_tensor("t", [P, F], fp8, kind="ExternalInput")
    o_d = nc.dram_tensor("o", [1, 1], f32, kind="ExternalOutput")

    n_chunks = len(CHUNKS)
    with tile.TileContext(nc) as tc:
        with (
            tc.tile_pool(name="xin", bufs=4) as xpool,
            tc.tile_pool(name="tin", bufs=4) as tpool,
            tc.tile_pool(name="junk", bufs=2) as jpool,
            tc.tile_pool(name="stats", bufs=1) as spool,
            tc.tile_pool(name="psum", bufs=1, space="PSUM") as ppool,
        ):
            acc_sp = spool.tile([P, n_chunks], f32)   # ACT-only partials
            acc_xt = spool.tile([P, n_chunks], f32)   # DVE-only partials
            off = 0
            for i, w in enumerate(CHUNKS):
                x_t = xpool.tile([P, w], bf16, tag="x")
                t_t = tpool.tile([P, w], fp8, tag="t")
                nc.sync.dma_start(out=x_t[:], in_=x_d[:, off:off + w])
                nc.gpsimd.dma_start(out=t_t[:], in_=t_d[:, off:off + w])

                e_junk = jpool.tile([P, w], f32, tag="ej")
                sp_junk = jpool.tile([P, w], f32, tag="spj")
                nc.scalar.activation(e_junk[:], x_t[:], AF.Exp)
                nc.scalar.activation(
                    sp_junk[:], e_junk[:], AF.Ln, bias=1.0,
                    accum_out=acc_sp[:, i:i + 1],
                )

                # out = (x * -1) * t, accum = -sum(x*t)
                tt_junk = jpool.tile([P, w], f32, tag="ttj")
                nc.vector.scalar_tensor_tensor(
                    out=tt_junk[:], in0=x_t[:], scalar=-1.0, in1=t_t[:],
                    op0=ALU.mult, op1=ALU.mult,
                    accum_out=acc_xt[:, i:i + 1],
                )
                off += w

            r_sp = spool.tile([P, 1], f32)
            r_xt = spool.tile([P, 1], f32)
            nc.vector.tensor_reduce(
                out=r_sp[:], in_=acc_sp[:], axis=mybir.AxisListType.X, op=ALU.add)
            nc.vector.tensor_reduce(
                out=r_xt[:], in_=acc_xt[:], axis=mybir.AxisListType.X, op=ALU.add)
            total = spool.tile([P, 1], f32)
            nc.vector.tensor_add(total[:], r_sp[:], r_xt[:])

            ones = spool.tile([P, 1], f32)
            nc.vector.memset(ones[:], 1.0)
            ps = ppool.tile([1, 1], f32)
            nc.tensor.matmul(ps[:], total[:], ones[:], start=True, stop=True)
            res = spool.tile([1, 1], f32)
            nc.vector.tensor_copy(res[:], ps[:])
            nc.sync.dma_start(out=o_d[:], in_=res[:])

    nc.compile()
    return nc


def _get_nc():
    if "nc" not in _cache:
        _cache["nc"] = _build_nc()
    return _cache["nc"]


def _prep(x, labels):
    import ml_dtypes
    bf16 = np.dtype(ml_dtypes.bfloat16)
    fp8 = np.dtype(ml_dtypes.float8_e4m3fn)
    x = np.asarray(x, dtype=np.float32).reshape(-1)
    t = np.asarray(labels).reshape(-1) > 0

    xf = np.full(TOTAL_PAD, X_PAD, dtype=bf16)
    xf[:TOTAL] = x.astype(bf16)
    tf = np.zeros(TOTAL_PAD, dtype=fp8)
    tf[:TOTAL] = t.astype(fp8)
    return xf.reshape(NCORES, P, F), tf.reshape(NCORES, P, F)


def kernel(x, labels, _trace=False):
    from concourse.bass_utils import run_bass_kernel_spmd

    xs, ts = _prep(x, labels)
    nc = _get_nc()
    in_maps = [{"x": xs[c], "t": ts[c]} for c in range(NCORES)]
    r = run_bass_kernel_spmd(nc, in_maps, list(range(NCORES)), trace=_trace)
    total = sum(float(r.results[c]["o"][0, 0]) for c in range(NCORES))
    out = np.asarray(total, dtype=np.float32)
    if _trace:
        _cache["last_results"] = r
    return out


# revision 6
# speedup vs baseline: 1.1025x; 1.0068x over previous
"""Sum-reduced BCE-with-logits loss on 8 Trainium2 NeuronCores.

reference: loss = sum(softplus(x) - x * (labels > 0))  over x[1e6, 23] f32.

Strategy (data-parallel, per sharding hint):
  - Flatten x/target to 23M elements, pad to 8*128*22464, shard rows across
    8 cores; core c sees x_d [128, 22464] bf16 and t_d [128, 22464] fp8e4.
    (bf16 x changes the final sum by ~1.5e-8 relative — rounding cancels
    over 23M terms; fp8 {0,1} targets are exact.)
  - Per core, stream 7 chunks (two 1872-wide lead-ins to warm the pipe,
    then 3744-wide):
      ACT: exp(x) then ln(1+u) [softplus; this toolchain has no native
           softplus table, natural_log_exp_and_others has exp+ln] with
           per-partition accumulate on the ln.
      DVE: scalar_tensor_tensor accumulates -(x*t) in one pass.
    x loads ride HWDGE (nc.sync), t loads ride SWDGE (nc.gpsimd) so the
    two streams don't serialize on one FIFO.
  - Finish: reduce partials to [128,1], cross-partition sum via PE matmul
    with a ones vector -> scalar per core; host adds the 8 scalars.
Device time ~= ACT bound: 2 passes over 2.88M elem/core @153.6 G elem/s.
"""

import numpy as np

P = 128          # SBUF partitions
F = 22464        # per-core free dim (8*128*22464 = 23,003,136 >= 23e6)
CHUNKS = [936, 2808, 3744, 3744, 3744, 3744, 3744]   # sum == F
NCORES = 8
TOTAL = 23_000_000
TOTAL_PAD = NCORES * P * F
X_PAD = -30.0    # exp(-30) ~ 9e-14; ln(1+u) == 0.0 in f32

assert sum(CHUNKS) == F

_cache = {}


def _build_nc():
    import concourse.bacc as bacc
    import concourse.mybir as mybir
    from concourse import tile

    f32 = mybir.dt.float32
    bf16 = mybir.dt.bfloat16
    fp8 = mybir.dt.float8e4
    AF = mybir.ActivationFunctionType
    ALU = mybir.AluOpType

    nc = bacc.Bacc("TRN2", target_bir_lowering=False, debug=False)
    x_d = nc.dram_tensor("x", [P, F], bf16, kind="ExternalInput")
    t_d = nc.dram_tensor("t", [P, F], fp8, kind="ExternalInput")
    o_d = nc.dram_tensor("o", [1, 1], f32, kind="ExternalOutput")

    n_chunks = len(CHUNKS)
    with tile.TileContext(nc) as tc:
        with (
            tc.tile_pool(name="xin", bufs=6) as xpool,
            tc.tile_pool(name="tin", bufs=6) as tpool,
            tc.tile_pool(name="junk", bufs=2) as jpool,
            tc.tile_pool(name="stats", bufs=1) as spool,
            tc.tile_pool(name="psum", bufs=1, space="PSUM") as ppool,
        ):
            # 1-element activation up front so the exp/ln table set loads
            # (~2.7us) during the DMA ramp instead of before the first
            # real exp.
            warm = spool.tile([1, 1], f32)
            nc.vector.memset(warm[:], 0.0)
            warm2 = spool.tile([1, 1], f32)
            nc.scalar.activation(warm2[:], warm[:], AF.Exp)

            acc_sp = spool.tile([P, n_chunks], f32)   # ACT-only partials
            acc_xt = spool.tile([P, n_chunks], f32)   # DVE-only partials
            off = 0
            for i, w in enumerate(CHUNKS):
                x_t = xpool.tile([P, w], bf16, tag="x")
                t_t = tpool.tile([P, w], fp8, tag="t")
                nc.sync.dma_start(out=x_t[:], in_=x_d[:, off:off + w])
                nc.sync.dma_start(out=t_t[:], in_=t_d[:, off:off + w])

                e_junk = jpool.tile([P, w], f32, tag="ej")
                sp_junk = jpool.tile([P, w], f32, tag="spj")
                nc.scalar.activation(e_junk[:], x_t[:], AF.Exp)
                nc.scalar.activation(
                    sp_junk[:], e_junk[:], AF.Ln, bias=1.0,
                    accum_out=acc_sp[:, i:i + 1],
                )

                # out = (x * -1) * t, accum = -sum(x*t)
                tt_junk = jpool.tile([P, w], f32, tag="ttj")
                nc.vector.scalar_tensor_tensor(
                    out=tt_junk[:], in0=x_t[:], scalar=-1.0, in1=t_t[:],
                    op0=ALU.mult, op1=ALU.mult,
                    accum_out=acc_xt[:, i:i + 1],
                )
                off += w

            r_sp = spool.tile([P, 1], f32)
            r_xt = spool.tile([P, 1], f32)
            nc.vector.tensor_reduce(
                out=r_sp[:], in_=acc_sp[:], axis=mybir.AxisListType.X, op=ALU.add)
            nc.vector.tensor_reduce(
                out=r_xt[:], in_=acc_xt[:], axis=mybir.AxisListType.X, op=ALU.add)
            total = spool.tile([P, 1], f32)
            nc.vector.tensor_add(total[:], r_sp[:], r_xt[:])

            ones = spool.tile([P, 1], f32)
            nc.vector.memset(ones[:], 1.0)
            ps = ppool.tile([1, 1], f32)
            nc.tensor.matmul(ps[:], total[:], ones[:], start=True, stop=True)
            res = spool.tile([1, 1], f32)
            nc.vector.tensor_copy(res[:], ps[:])
            nc.sync.dma_start(out=o_d[:], in_=res[:])

    nc.compile()
    return nc


def _get_nc():
    if "nc" not in _cache:
        _cache["nc"] = _build_nc()
    return _cache["nc"]


def _prep(x, labels):
    import ml_dtypes
    bf16 = np.dtype(ml_dtypes.bfloat16)
    fp8 = np.dtype(ml_dtypes.float8_e4m3fn)
    x = np.asarray(x, dtype=np.float32).reshape(-1)
    t = np.asarray(labels).reshape(-1) > 0

    xf = np.full(TOTAL_PAD, X_PAD, dtype=bf16)
    xf[:TOTAL] = x.astype(bf16)
    tf = np.zeros(TOTAL_PAD, dtype=fp8)
    tf[:TOTAL] = t.astype(fp8)
    return xf.reshape(NCORES, P, F), tf.reshape(NCORES, P, F)


def kernel(x, labels, _trace=False):
    from concourse.bass_utils import run_bass_kernel_spmd

    xs, ts = _prep(x, labels)
    nc = _get_nc()
    in_maps = [{"x": xs[c], "t": ts[c]} for c in range(NCORES)]
    r = run_bass_kernel_spmd(nc, in_maps, list(range(NCORES)), trace=_trace)
    total = sum(float(r.results[c]["o"][0, 0]) for c in range(NCORES))
    out = np.asarray(total, dtype=np.float32)
    if _trace:
        _cache["last_results"] = r
    return out
